# revision 13
# baseline (speedup 1.0000x reference)
"""Trainium2 Bass kernel for nn_CIN (3-layer CIN / xDeepFM feature-interaction).

Reference computation per layer k (x: (B,39,16), h0 = x):
    z[b,f,g,d] = x[b,f,d] * h[b,g,d]
    cur[b,l,d] = relu(sum_{f,g} z[b,f,g,d] * Wk[f*Fk+g, l] + bk[l])
    h <- cur[:, :64] (layers 0,1);  direct outputs concat'd, summed over d.

Sharding: pure data parallelism, batch 1024 -> 8 cores x 128 rows.

Device layout per core: everything is (partition, n) with n = b*16+d in [0,2048).
The (f,g) interaction pairs are covered by K-chunks of 128 pairs, each either:
  mult path:   DMA: xf = x rows replicated to the chunk's (f,g) rows (SBUF)
               DVE: z = xf * h_rep          (SBUF x SBUF bf16 -> 2x mode)
  square path: PE:  P = Ssq_c^T @ [x; h]    (x_f - h_g per row, PSUM)
               ACT: z = Square(P)           (x*h = -(1/2)((x-h)^2 - x^2 - h^2);
                                             signs/corrections folded into W)
  both:        PE:  cur += Wc_c^T @ z       (accumulating matmul, PSUM)
Square-path residuals (.5*w*x_f^2 + .5*w*h_g^2) are one extra K-chunk per layer
of host-folded weights against Square([x; h]).

Layer 0 exploits x (x) x symmetry: f<SPLIT0 rows go through the mult path with
original ordered weights; the remaining ordered pairs fold onto unordered pairs
(w = W[a,b]+W[b,a], or the single missing order) handled by the square path,
whose selector columns are unconstrained; diagonals fold into the correction.
This cuts layer-0 chunks 13 -> 10.

All matmul operands are bf16 (fp32 PSUM accumulate).
"""

import numpy as np

B, F, D, L = 1024, 39, 16, 128
NCORES = 8
BC = B // NCORES          # 128 batch rows per core
NF = BC * D               # 2048 free elements per core
HALF = NF // 2            # 1024: psum-bank-pair granule
KP = 128                  # chunk height (partitions)
SPLIT0 = 15               # layer-0: f < SPLIT0 handled by mult path
N_WARM = 24               # PE warm-up matmuls at kernel start

_CACHE = {}


def _plan():
    """Structural chunk plan (no weight values), shared by host + device.

    mult: {kind:'mult', fbase, nf}          (l0: f-triples; l1/2: f-pairs)
    sq:   {kind:'sq', pairs: [(urow, vrow, f, g, mode)]}
        urow/vrow: row indices in the bcast rhs tile (l0: xT; l1/2: xh)
        mode 'one' -> W[f,g];  'sym' -> W[f,g] + W[g,f]
    """
    layers = []
    # ---- layer 0: 5 mult (f<15) + 5 sq (folded remainder), interleaved ----
    mult = [{"kind": "mult", "fbase": 3 * i, "nf": 3} for i in range(5)]
    entries = []
    for a in range(SPLIT0):
        for b in range(SPLIT0, 39):
            entries.append((b, a, b, a, "one"))       # missing order (f=b, g=a)
    for a in range(SPLIT0, 39):
        for b in range(a + 1, 39):
            entries.append((a, b, a, b, "sym"))
    sq = [
        {"kind": "sq", "pairs": entries[i : i + KP]}
        for i in range(0, len(entries), KP)
    ]
    order = []
    for i in range(max(len(mult), len(sq))):
        if i < len(mult):
            order.append(mult[i])
        if i < len(sq):
            order.append(sq[i])
    layers.append(order)
    # ---- layers 1, 2: 20 f-pair chunks, 6 on the square path ----
    order = []
    nsq = 0
    for i in range(20):
        fb = 2 * i
        nf = 1 if fb == 38 else 2
        if i % 3 == 2 and nsq < 6:
            nsq += 1
            order.append({
                "kind": "sq",
                "pairs": [
                    (f, 39 + g, f, g, "one")
                    for f in range(fb, min(fb + nf, 39))
                    for g in range(64)
                ],
            })
        else:
            order.append({"kind": "mult", "fbase": fb, "nf": nf})
    layers.append(order)
    layers.append(order)
    return layers


PLAN = _plan()
NCH = [len(p) for p in PLAN]


def _mult_rows(layer, c):
    """(tile_row, f, g) triples for a mult chunk's 128 z-rows (f>=39 = pad)."""
    fk = 39 if layer == 0 else 64
    out = []
    for p in range(KP):
        f = c["fbase"] + p // fk
        if p // fk >= c["nf"] or f >= 39:
            out.append((p, 39, 0))
        else:
            out.append((p, f, p % fk))
    return out


def _host_consts(W0, W1, W2):
    """Fold reference weights into device constant tensors (fp32, cast later)."""
    Ws = (W0.reshape(39, 39, L), W1.reshape(39, 64, L), W2.reshape(39, 64, L))
    out = {}
    corr_all = np.zeros((103, 3 * L), np.float32)
    for layer in (0, 1, 2):
        W = Ws[layer]
        nch = NCH[layer]
        wc = np.zeros((KP, nch * L), np.float32)
        corr = corr_all[:, layer * L : (layer + 1) * L]
        if layer == 0:
            for a in range(SPLIT0, 39):     # diagonal x_a^2 terms, a >= SPLIT0
                corr[a] += W[a, a]
        if layer < 2:
            ssq_rows = 39 if layer == 0 else 103
            ssq = np.zeros((ssq_rows, nch * KP), np.float32)
        for ci, c in enumerate(PLAN[layer]):
            if c["kind"] == "mult":
                for p, f, g in _mult_rows(layer, c):
                    if f >= 39:
                        continue
                    wc[p, ci * L : (ci + 1) * L] = W[f, g]
            else:
                for p, (ur, vr, f, g, mode) in enumerate(c["pairs"]):
                    w = W[f, g] + (W[g, f] if mode == "sym" else 0.0)
                    wc[p, ci * L : (ci + 1) * L] = -0.5 * w
                    corr[ur] += 0.5 * w
                    corr[vr] += 0.5 * w
                    if layer < 2:
                        ssq[ur, ci * KP + p] += 1.0
                        ssq[vr, ci * KP + p] -= 1.0
        out[f"Wc{layer}"] = wc
        if layer < 2:
            out[f"Ssq{layer}"] = ssq
    out["corr"] = corr_all
    return out


def _build_nc():
    import concourse.bacc as bacc
    import concourse.tile as tile
    from concourse import bass, mybir

    F32 = mybir.dt.float32
    BF16 = mybir.dt.bfloat16
    nc = bacc.Bacc("TRN2", target_bir_lowering=False, debug=False, num_devices=NCORES)

    dram = {}

    def din(name, shape, dt=BF16):
        dram[name] = nc.dram_tensor(name, shape, dt, kind="ExternalInput").ap()

    # declaration order == DMA issue order: layer-0 critical first
    din("xT", (39, NF))
    din("Ssq0", (39, NCH[0] * KP))
    din("Wc0", (KP, NCH[0] * L))
    din("corr", (103, 3 * L))
    din("bias", (L, 3), dt=F32)
    din("xfall0", (KP, 5 * NF))
    din("xT3", (KP, NF))
    din("xfall12", (KP, 14 * NF))
    din("Ssq1", (103, NCH[1] * KP))
    din("Wc1", (KP, NCH[1] * L))
    din("Wc2", (KP, NCH[2] * L))
    out_d = nc.dram_tensor("out", (256, BC), F32, kind="ExternalOutput").ap()

    with tile.TileContext(nc) as tc:
        with (
            tc.tile_pool(name="const", bufs=1) as cp,
            tc.tile_pool(name="work", bufs=2) as wp,
            tc.tile_pool(name="relu", bufs=1) as rp,
            tc.tile_pool(name="zp", bufs=6) as zp,
            tc.tile_pool(name="pbc", bufs=2, space="PSUM") as pbc,
            tc.tile_pool(name="pcur", bufs=1, space="PSUM") as pcur,
        ):
            ct = {}
            for name in dram:
                if name == "out":
                    continue
                ct[name] = cp.tile(
                    list(dram[name].shape), dram[name].dtype, tag=name, name=f"c_{name}"
                )
                nc.sync.dma_start(out=ct[name], in_=dram[name])

            # PE warm-up while constants stream in (HAM needs ~3.4us busy).
            for wi in range(N_WARM):
                wt = pbc.tile([KP, HALF], F32, tag="bc", name=f"warm{wi}")
                nc.tensor.matmul(
                    wt[:, 0:512],
                    lhsT=ct["Ssq0"][0:39, 0:KP],
                    rhs=ct["xT"][0:39, 0:512],
                    start=True,
                    stop=True,
                )

            relu_h = [None] * 3
            relu_d = [None] * 3
            red_t = [None] * 3
            xh = [None] * 3
            xhsq = [None] * 3
            xh[0] = ct["xT"]

            for layer in (0, 1, 2):
                wc = ct[f"Wc{layer}"]
                sq_rows = 39 if layer == 0 else 103
                ssq = ct["Ssq0"] if layer == 0 else ct["Ssq1"]

                if layer == 0:
                    h_rep = ct["xT3"]
                    xrep_src = ct["xT"]
                else:
                    prev = relu_h[layer - 1]
                    h_rep = wp.tile([128, NF], BF16, tag="h_rep")
                    nc.sync.dma_start(out=h_rep[0:64, :], in_=prev[0:64, :])
                    nc.sync.dma_start(out=h_rep[64:128, :], in_=prev[0:64, :])
                    xh[layer] = wp.tile([103, NF], BF16, tag="xh", name=f"xh{layer}")
                    nc.sync.dma_start(out=xh[layer][0:39, :], in_=dram["xT"])
                    nc.sync.dma_start(out=xh[layer][39:103, :], in_=prev[0:64, :])
                    xrep_src = ct["xT"]

                xhsq[layer] = wp.tile(
                    [sq_rows, NF], BF16, tag="xhsq", name=f"xhsq{layer}"
                )
                nc.scalar.activation(
                    out=xhsq[layer][:, :],
                    in_=xh[layer][0:sq_rows, :],
                    func=mybir.ActivationFunctionType.Square,
                )

                cur = pcur.tile([128, NF], F32, tag="cur")
                xfall = ct["xfall0"] if layer == 0 else ct["xfall12"]
                mi = 0
                for ci, c in enumerate(PLAN[layer]):
                    if c["kind"] == "mult":
                        xf = xfall[:, mi * NF : (mi + 1) * NF]
                        mi += 1
                        zt = zp.tile([KP, NF], BF16, tag="z", name=f"zm{layer}_{ci}")
                        nc.vector.tensor_mul(zt[:, :], xf[:, :], h_rep[0:KP, :])
                        for q in range(4):
                            qs = slice(q * 512, (q + 1) * 512)
                            nc.tensor.matmul(
                                cur[:, qs],
                                lhsT=wc[:, ci * L : (ci + 1) * L],
                                rhs=zt[:, qs],
                                start=(ci == 0),
                                stop=False,
                            )
                    else:
                        for half in range(2):
                            bc = pbc.tile([KP, HALF], F32, tag="bc")
                            for q in range(2):
                                qs = slice(q * 512, (q + 1) * 512)
                                nqs = slice(
                                    half * HALF + q * 512, half * HALF + (q + 1) * 512
                                )
                                nc.tensor.matmul(
                                    bc[:, qs],
                                    lhsT=ssq[0:sq_rows, ci * KP : (ci + 1) * KP],
                                    rhs=xh[layer][0:sq_rows, nqs],
                                    start=True,
                                    stop=True,
                                )
                            zt = zp.tile(
                                [KP, HALF], BF16, tag="zs", name=f"zs{layer}_{ci}"
                            )
                            nc.scalar.activation(
                                out=zt[:, :],
                                in_=bc[:, :],
                                func=mybir.ActivationFunctionType.Square,
                            )
                            for q in range(2):
                                qs = slice(q * 512, (q + 1) * 512)
                                nqs = slice(
                                    half * HALF + q * 512, half * HALF + (q + 1) * 512
                                )
                                nc.tensor.matmul(
                                    cur[:, nqs],
                                    lhsT=wc[:, ci * L : (ci + 1) * L],
                                    rhs=zt[:, qs],
                                    start=(ci == 0),
                                    stop=False,
                                )

                # correction chunk (always last accumulation into each bank)
                corr = ct["corr"][0:sq_rows, layer * L : (layer + 1) * L]
                for q in range(4):
                    qs = slice(q * 512, (q + 1) * 512)
                    nc.tensor.matmul(
                        cur[:, qs],
                        lhsT=corr,
                        rhs=xhsq[layer][:, qs],
                        start=False,
                        stop=True,
                    )

                # relu: h-half first (critical path), direct half after
                bias_ap = ct["bias"][:, layer : layer + 1]
                if layer < 2:
                    relu_h[layer] = rp.tile(
                        [64, NF], BF16, tag=f"rh{layer}", name=f"rh{layer}"
                    )
                    nc.scalar.activation(
                        out=relu_h[layer][:, :],
                        in_=cur[0:64, :],
                        func=mybir.ActivationFunctionType.Relu,
                        bias=bias_ap[0:64],
                        scale=1.0,
                    )
                    relu_d[layer] = rp.tile(
                        [64, NF], BF16, tag=f"rd{layer}", name=f"rd{layer}"
                    )
                    nc.vector.tensor_scalar(
                        out=relu_d[layer][:, :],
                        in0=cur[64:128, :],
                        scalar1=bias_ap[64:128],
                        scalar2=0.0,
                        op0=mybir.AluOpType.add,
                        op1=mybir.AluOpType.max,
                    )
                else:
                    relu_d[layer] = rp.tile(
                        [128, NF], BF16, tag=f"rd{layer}", name=f"rd{layer}"
                    )
                    nc.scalar.activation(
                        out=relu_d[layer][:, :],
                        in_=cur[:, :],
                        func=mybir.ActivationFunctionType.Relu,
                        bias=bias_ap,
                        scale=1.0,
                    )

                nr = 64 if layer < 2 else 128
                red_t[layer] = rp.tile(
                    [nr, BC], F32, tag=f"red{layer}", name=f"red{layer}"
                )
                nc.vector.tensor_reduce(
                    out=red_t[layer][:, :],
                    in_=relu_d[layer].rearrange("p (b d) -> p b d", d=D),
                    axis=mybir.AxisListType.X,
                    op=mybir.AluOpType.add,
                )

            nc.sync.dma_start(out=out_d[0:64, :], in_=red_t[0])
            nc.sync.dma_start(out=out_d[64:128, :], in_=red_t[1])
            nc.sync.dma_start(out=out_d[128:256, :], in_=red_t[2])

    nc.compile()
    return nc


def _get_nc():
    if "nc" not in _CACHE:
        _CACHE["nc"] = _build_nc()
    return _CACHE["nc"]


def _install_profile_shim():
    import sys, types

    if "antenv.axon_hooks" in sys.modules:
        return
    try:
        from trn_agent_boot.trn_boot import _ntff_profile_via_ctypes

        hook = _ntff_profile_via_ctypes("/opt/axon/libaxon_pjrt.so")
    except Exception:
        hook = None
    m = types.ModuleType("antenv.axon_hooks")
    m.get_axon_ntff_profile_hook = lambda: hook
    sys.modules["antenv.axon_hooks"] = m


def _to_bf16(a):
    import ml_dtypes

    return np.ascontiguousarray(a).astype(ml_dtypes.bfloat16)


def host_in_maps(inputs):
    """Host-side sharding + constant folding -> per-core device input maps."""
    x = np.asarray(inputs["x"], np.float32)
    consts = _host_consts(
        np.asarray(inputs["W0"], np.float32),
        np.asarray(inputs["W1"], np.float32),
        np.asarray(inputs["W2"], np.float32),
    )
    consts = {k: _to_bf16(v) for k, v in consts.items()}
    bias = np.stack(
        [np.asarray(inputs[f"b{i}"], np.float32) for i in range(3)], axis=1
    )  # (128, 3)

    in_maps = []
    for c in range(NCORES):
        xT = _to_bf16(x[c * BC : (c + 1) * BC].transpose(1, 0, 2).reshape(39, NF))
        xf0 = np.zeros((KP, 5 * NF), xT.dtype)
        xf12 = np.zeros((KP, 14 * NF), xT.dtype)
        for layer, xf in ((0, xf0), (1, xf12)):
            mi = 0
            for c in PLAN[layer]:
                if c["kind"] != "mult":
                    continue
                for p, f, g in _mult_rows(layer, c):
                    xf[p, mi * NF : (mi + 1) * NF] = xT[f if f < 39 else 0]
                mi += 1
        m = {
            "xT": xT,
            "xT3": np.ascontiguousarray(np.tile(xT, (4, 1))[:KP]),
            "xfall0": xf0,
            "xfall12": xf12,
            "bias": np.ascontiguousarray(bias),
        }
        m.update(consts)
        in_maps.append(m)
    return in_maps


def run(inputs, trace=False, trace_cores=None):
    """Run the SPMD kernel; returns (out (1024,256) fp32, BassKernelResults)."""
    from concourse.bass_utils import run_bass_kernel_spmd

    _install_profile_shim()
    in_maps = host_in_maps(inputs)
    nc = _get_nc()
    res = run_bass_kernel_spmd(
        nc, in_maps, list(range(NCORES)), trace=trace, trace_cores=trace_cores
    )
    out = np.concatenate(
        [res.results[c]["out"].T for c in range(NCORES)], axis=0
    ).astype(np.float32)
    return out, res


def kernel(**inputs):
    out, _ = run(inputs, trace=False)
    return out
